# revision 1
# baseline (speedup 1.0000x reference)
"""BiEncoder (bidirectional LSTM over video features) Trainium2 kernel.

Sharding: 8 NeuronCores = 4 batch groups (B=64 each) x 2 directions.
Every core runs the SAME program (SPMD); the host hands backward-direction
cores time-reversed inputs and the direction's weights, and un-reverses the
outputs.

Per-core program:
  phase A (per 8-step chunk): embed  v = video @ W_e.T + b_e   (fp16 matmul)
                              xg     = v @ W_ih_s.T + b_s      (fp16 matmul)
  phase B (scan, 64 steps):   hg     = W_hh_s @ h_prev         (fp16 matmul)
                              t      = tanh(hg + xg)           (one ACT op)
                              c      = (t_f+1)/2*c + (t_i+1)/2*t_g
                              h      = (t_o+1)/2*tanh(c)
  using sigmoid(x) = (tanh(x/2)+1)/2 with the 1/2 folded into the i/f/o
  rows of W_ih/W_hh/bias on the host, so ONE tanh instruction covers all
  four gate groups.
"""

import sys
import time

for _p in ("/opt/trn_rl_repo", "/root/.axon_site/_ro/trn_rl_repo"):
    if _p not in sys.path:
        sys.path.insert(0, _p)

import numpy as np
import jax

try:
    # persistent XLA compile cache: a fresh process re-running this exact
    # kernel skips the multi-minute walrus/neuronx compile
    jax.config.update("jax_compilation_cache_dir", "/tmp/jax_cc_cache")
    jax.config.update("jax_persistent_cache_min_entry_size_bytes", 0)
    jax.config.update("jax_persistent_cache_min_compile_time_secs", 0.0)
except Exception:
    pass

import concourse.tile as tile
from concourse import bacc, mybir
from concourse.bass import ts
from concourse.bass_utils import run_bass_kernel_spmd

F16 = mybir.dt.float16
F32 = mybir.dt.float32
F8 = mybir.dt.float8e4
AF = mybir.ActivationFunctionType
OP = mybir.AluOpType

# phase A (embed + input projections) in fp8e4m3 with DoubleRow K-packing:
# halves both the PE time and the instruction count of phase A. Weights are
# pre-scaled x16 into fp8's normal range; the 1/16 is folded into the
# activation's free scale. DISABLED: DoubleRow gave ~16% error on hardware
# (interleave semantics differ from the [K,2,N] AP used here) — fp16 path
# is the verified configuration.
FP8A = False
F8_NP = mybir.dt.np(F8)
W8SCALE = 16.0

B, T, F, P, H = 256, 64, 2048, 512, 512
NB = 4          # batch groups
BC = B // NB    # 64 per-core batch
TC = 8          # timesteps per phase-A chunk
NCHUNK = T // TC
KF = F // 128   # 16  F tiles
KP = P // 128   # 4   P tiles
KH = H // 128   # 4   H tiles
MG = 4 * H // 128  # 16 gate tiles


def build_nc():
    nc = bacc.Bacc("TRN2", target_bir_lowering=False, debug=False, num_devices=8)

    # all layouts partition-major so every DMA is one long contiguous run
    # per partition (minimizes DMA descriptor count)
    FA = F8 if FP8A else F16
    vt_d = nc.dram_tensor("vt", [NCHUNK, 128, KF, TC, BC], FA, kind="ExternalInput")
    wet_d = nc.dram_tensor("w_et", [128, KF, P], FA, kind="ExternalInput")
    bet_d = nc.dram_tensor("b_e_t", [128, KP], F32, kind="ExternalInput")
    wih_d = nc.dram_tensor("w_iht", [128, KP, 4 * H], FA, kind="ExternalInput")
    whh_d = nc.dram_tensor("w_hht", [128, KH, 4 * H], F16, kind="ExternalInput")
    bias_d = nc.dram_tensor("bias", [128, MG], F32, kind="ExternalInput")
    out_d = nc.dram_tensor("out_h", [NCHUNK, 128, TC, KH, BC], F16, kind="ExternalOutput")

    with tile.TileContext(nc) as tc:
        with (
            tc.tile_pool(name="const", bufs=1) as const,
            tc.tile_pool(name="vload", bufs=3) as vload,
            tc.tile_pool(name="vtp", bufs=3) as vtp,
            tc.tile_pool(name="xchunk", bufs=3) as xchunk,
            tc.tile_pool(name="state", bufs=3) as state,
            tc.tile_pool(name="tmp", bufs=2) as tmp,
            tc.tile_pool(name="psv", bufs=2, space="PSUM") as psv,
            tc.tile_pool(name="psx", bufs=2, space="PSUM") as psx,
            tc.tile_pool(name="psg", bufs=2, space="PSUM") as psg,
        ):
            # resident weights
            wet = const.tile([128, KF, P], FA)
            nc.sync.dma_start(wet[:], wet_d.ap())
            wih = const.tile([128, KP, 4 * H], FA)
            nc.sync.dma_start(wih[:], wih_d.ap())
            whh = const.tile([128, KH, 4 * H], F16)
            nc.sync.dma_start(whh[:], whh_d.ap())
            bet = const.tile([128, KP], F32)
            nc.sync.dma_start(bet[:], bet_d.ap())
            bias = const.tile([128, MG], F32)
            nc.sync.dma_start(bias[:], bias_d.ap())

            # initial state
            h_prev = state.tile([128, KH, BC], F16, tag="h")
            nc.gpsimd.memset(h_prev[:], 0.0)
            c_prev = state.tile([128, KH, BC], F32, tag="c")
            nc.gpsimd.memset(c_prev[:], 0.0)

            def phase_a_items(c):
                """Yield phase-A work items (closures) for chunk c; the
                caller interleaves them between scan steps of chunk c-1.
                First item returns the xgc tile."""
                vch = vload.tile([128, KF, TC * BC], FA, tag="vch")
                vsb = vtp.tile([128, KP, TC * BC], FA, tag="vsb")
                xgc = xchunk.tile([128, TC, MG, BC], F16, tag="xgc")
                a_scale = (1.0 / W8SCALE) if FP8A else 1.0
                pm = mybir.MatmulPerfMode.DoubleRow if FP8A else None

                def dma_item():
                    nc.sync.dma_start(
                        vch[:], vt_d.ap()[c].rearrange("p ko t b -> p ko (t b)")
                    )

                def embed_item(mp):
                    pv = psv.tile([128, TC * BC], F32, tag="pv")
                    if FP8A:
                        for k2 in range(KF // 2):
                            nc.tensor.matmul(
                                pv[:],
                                wet[:, 2 * k2 : 2 * k2 + 2, ts(mp, 128)],
                                vch[:, 2 * k2 : 2 * k2 + 2, :],
                                start=(k2 == 0),
                                stop=(k2 == KF // 2 - 1),
                                perf_mode=pm,
                            )
                    else:
                        for ko in range(KF):
                            nc.tensor.matmul(
                                pv[:],
                                wet[:, ko, ts(mp, 128)],
                                vch[:, ko, :],
                                start=(ko == 0),
                                stop=(ko == KF - 1),
                            )
                    nc.scalar.activation(
                        vsb[:, mp, :], pv[:], AF.Identity,
                        bias=bet[:, mp : mp + 1], scale=a_scale,
                    )

                def xg_item(m0):
                    for m in range(m0, m0 + 2):
                        px = psx.tile([128, TC, BC], F32, tag="px")
                        if FP8A:
                            for k2 in range(KP // 2):
                                nc.tensor.matmul(
                                    px[:],
                                    wih[:, 2 * k2 : 2 * k2 + 2, ts(m, 128)],
                                    vsb[:, 2 * k2 : 2 * k2 + 2, :],
                                    start=(k2 == 0),
                                    stop=(k2 == KP // 2 - 1),
                                    perf_mode=pm,
                                )
                        else:
                            for kp in range(KP):
                                nc.tensor.matmul(
                                    px[:],
                                    wih[:, kp, ts(m, 128)],
                                    vsb[:, kp, :],
                                    start=(kp == 0),
                                    stop=(kp == KP - 1),
                                )
                        nc.scalar.activation(
                            xgc[:, :, m, :], px[:], AF.Identity,
                            bias=bias[:, m : m + 1], scale=a_scale,
                        )

                items = [dma_item]
                items += [lambda mp=mp: embed_item(mp) for mp in range(KP)]
                items += [lambda m0=m0: xg_item(m0) for m0 in range(0, MG, 2)]
                return xgc, items

            def scan_step(t, xgc, tl, hstage):
                nonlocal h_prev, c_prev
                # gate-tile order is [i, g, f, o] (host permutes the weights):
                # PE group 0 = {i,g} so u_i*t_g computes during groups 1/2;
                # group 1 = {f} so the c chain starts during group 2 = {o}.
                pg = psg.tile([128, MG, BC], F32, tag="pg")
                th = tmp.tile([128, MG, BC], F16, tag="th")
                groups = [(0, 2 * KH), (2 * KH, 3 * KH), (3 * KH, MG)]

                def mm_group(lo, hi):
                    for m in range(lo, hi):
                        for kh in range(KH):
                            nc.tensor.matmul(
                                pg[:, m, :],
                                whh[:, kh, ts(m, 128)],
                                h_prev[:, kh, :],
                                start=(kh == 0),
                                stop=(kh == KH - 1),
                            )

                def add_tanh(lo, hi):
                    gs = tmp.tile([128, hi - lo, BC], F16, tag=f"gs{lo}")
                    nc.vector.tensor_tensor(
                        gs[:], pg[:, lo:hi, :], xgc[:, tl, lo:hi, :], OP.add
                    )
                    nc.scalar.activation(th[:, lo:hi, :], gs[:], AF.Tanh)

                # group 0: i, g
                mm_group(*groups[0])
                add_tanh(*groups[0])
                # group 1 matmuls (f) — overlap with u_i*t_g below
                mm_group(*groups[1])
                ui = tmp.tile([128, KH, BC], F32, tag="ui")
                nc.vector.tensor_scalar(ui[:], th[:, :KH, :], 1.0, 0.5, OP.add, OP.mult)
                m2 = tmp.tile([128, KH, BC], F32, tag="m2")
                nc.vector.tensor_tensor(m2[:], ui[:], th[:, KH : 2 * KH, :], OP.mult)
                add_tanh(*groups[1])
                # group 2 matmuls (o) — overlap with the c chain below
                mm_group(*groups[2])
                uf = tmp.tile([128, KH, BC], F32, tag="uf")
                nc.vector.tensor_scalar(
                    uf[:], th[:, 2 * KH : 3 * KH, :], 1.0, 0.5, OP.add, OP.mult
                )
                m1 = tmp.tile([128, KH, BC], F32, tag="m1")
                nc.vector.tensor_tensor(m1[:], uf[:], c_prev[:], OP.mult)
                c_new = state.tile([128, KH, BC], F32, tag="c")
                nc.vector.tensor_tensor(c_new[:], m1[:], m2[:], OP.add)
                tc_t = tmp.tile([128, KH, BC], F16, tag="tct")
                nc.scalar.activation(tc_t[:], c_new[:], AF.Tanh)
                add_tanh(*groups[2])
                uo = tmp.tile([128, KH, BC], F16, tag="uo")
                nc.vector.tensor_scalar(
                    uo[:], th[:, 3 * KH :, :], 1.0, 0.5, OP.add, OP.mult
                )
                h_new = hstage[:, tl, :, :]
                nc.vector.tensor_tensor(h_new, uo[:], tc_t[:], OP.mult)
                h_prev, c_prev = h_new, c_new

            # software-pipelined emission: phase A of chunk c+1 interleaves
            # with the scan steps of chunk c
            xgc_cur, items = phase_a_items(0)
            for it in items:
                it()
            for c in range(NCHUNK):
                if c + 1 < NCHUNK:
                    xgc_next, items = phase_a_items(c + 1)
                else:
                    xgc_next, items = None, []
                hstage = state.tile([128, TC, KH, BC], F16, tag="hs")
                n_items = len(items)
                emitted = 0
                for tl in range(TC):
                    scan_step(c * TC + tl, xgc_cur, tl, hstage)
                    want = (n_items * (tl + 1)) // TC
                    while emitted < want:
                        items[emitted]()
                        emitted += 1
                nc.sync.dma_start(out_d.ap()[c], hstage[:])
                xgc_cur = xgc_next

    nc.compile()
    return nc


_CACHED_NC = None


def _get_nc():
    global _CACHED_NC
    if _CACHED_NC is None:
        _CACHED_NC = build_nc()
    return _CACHED_NC


def _prep_inputs(video_feats, W_e, b_e, W_ih1, W_hh1, b_ih1, b_hh1,
                 W_ih2, W_hh2, b_ih2, b_hh2):
    """Build the 8 per-core input maps (host-side shard + layout prep)."""
    # gate scaling: i, f, o rows get 0.5 (sigmoid-via-tanh); g rows 1.0.
    # gate rows are permuted [i, f, g, o] -> [i, g, f, o] to match the
    # kernel's PE group order.
    s = np.ones((4 * H,), np.float32)
    s[0 * H : 2 * H] = 0.5
    s[3 * H : 4 * H] = 0.5
    perm = np.concatenate(
        [
            np.arange(0 * H, 1 * H),  # i
            np.arange(2 * H, 3 * H),  # g
            np.arange(1 * H, 2 * H),  # f
            np.arange(3 * H, 4 * H),  # o
        ]
    )

    if FP8A:
        a_np, w_escale = F8_NP, W8SCALE
    else:
        a_np, w_escale = np.float16, 1.0
    wet = np.ascontiguousarray(
        (W_e.T * w_escale).astype(a_np).reshape(KF, 128, P).transpose(1, 0, 2)
    )
    bet = np.ascontiguousarray(b_e.reshape(KP, 128).T).astype(np.float32)

    per_dir = []
    for (W_ih, W_hh, b_ih, b_hh) in (
        (W_ih1, W_hh1, b_ih1, b_hh1),
        (W_ih2, W_hh2, b_ih2, b_hh2),
    ):
        wih = (((W_ih * s[:, None])[perm]).T * w_escale).astype(a_np)
        whh = ((W_hh * s[:, None])[perm]).T.astype(np.float16)
        bb = (((b_ih + b_hh) * s)[perm]).astype(np.float32)
        per_dir.append(
            (
                np.ascontiguousarray(wih.reshape(KP, 128, 4 * H).transpose(1, 0, 2)),
                np.ascontiguousarray(whh.reshape(KH, 128, 4 * H).transpose(1, 0, 2)),
                np.ascontiguousarray(bb.reshape(MG, 128).T),
            )
        )

    # videoT [F, T, B]
    vt_full = np.ascontiguousarray(video_feats.transpose(2, 1, 0)).astype(a_np)
    vt_rev = np.ascontiguousarray(vt_full[:, ::-1, :])

    in_maps = []
    for core in range(8):
        g, d = divmod(core, 2)
        src = vt_full if d == 0 else vt_rev
        # [F,T,Bc] -> [NCHUNK, 128, KF, TC, BC]
        vt = np.ascontiguousarray(
            src[:, :, g * BC : (g + 1) * BC]
            .reshape(KF, 128, NCHUNK, TC, BC)
            .transpose(2, 1, 0, 3, 4)
        )
        wih, whh, bb = per_dir[d]
        in_maps.append(
            {
                "vt": vt,
                "w_et": wet,
                "b_e_t": bet,
                "w_iht": wih,
                "w_hht": whh,
                "bias": bb,
            }
        )
    return in_maps


last_exec_ns = None
last_wall_s = None


def kernel(**inputs):
    global last_exec_ns, last_wall_s
    nc = _get_nc()
    inputs = {k: np.asarray(v, dtype=np.float32) for k, v in inputs.items()}
    in_maps = _prep_inputs(**inputs)
    t0 = time.perf_counter()
    res = run_bass_kernel_spmd(nc, in_maps, core_ids=list(range(8)))
    last_wall_s = time.perf_counter() - t0
    last_exec_ns = res.exec_time_ns

    lstm1 = np.empty((B, T, H), np.float32)
    lstm2 = np.empty((B, T, H), np.float32)
    for core in range(8):
        g, d = divmod(core, 2)
        oh = res.results[core]["out_h"]  # [NCHUNK, 128, TC, KH, BC] f16
        h = np.transpose(oh.astype(np.float32), (4, 0, 2, 3, 1)).reshape(BC, T, H)
        if d == 0:
            lstm1[g * BC : (g + 1) * BC] = h
        else:
            lstm2[g * BC : (g + 1) * BC] = h[:, ::-1, :]
    return (lstm1, lstm2)



# revision 45
# speedup vs baseline: 1.6940x; 1.6940x over previous
"""BiEncoder (bidirectional LSTM over video features) Trainium2 kernel.

Sharding: 8 NeuronCores = 4 batch groups (B=64 each) x 2 directions (SPMD:
backward cores get time-reversed inputs + that direction's weights; the host
un-reverses outputs).

Per-core program (v2):
  - Each core's 64-batch is split into TWO independent 32-batch chains that
    interleave, hiding the per-step recurrence latency.
  - Phase A (per 8-step chunk): embed v = W_e @ x in fp8 DoubleRow matmuls,
    PSUM drained to SBUF (fp8) by the Vector engine.
  - x-projections W_ih @ v + b accumulate DIRECTLY into the scan step's PSUM
    tile two steps ahead (fp8 DoubleRow; the gate bias enters via an extra
    DoubleRow pair against a constant one-hot rhs).
  - Scan step (per chain): W_hh @ h8 accumulates on top (fp8 DoubleRow),
    then: act1 = tanh over [i,g] tiles (i rows pre-halved on the host so one
    tanh covers sigmoid-via-tanh for i and the real tanh for g), act2 = real
    Sigmoid over [f,o], c = s_f*c + 0.5(th_i+1)*th_g, h8 = s_o*tanh(c) (fp8
    for the recurrence), h16 = same in f16 (for output).
  - Gate order is permuted to [i, g, f, o] on the host; all fp8 weights are
    scaled x16 (x8 for the pre-halved i rows) with 1/16 folded into the
    activation scale.
"""

import sys
import time

for _p in ("/opt/trn_rl_repo", "/root/.axon_site/_ro/trn_rl_repo"):
    if _p not in sys.path:
        sys.path.insert(0, _p)

import numpy as np
import jax

try:
    jax.config.update("jax_compilation_cache_dir", "/tmp/jax_cc_cache")
    jax.config.update("jax_persistent_cache_min_entry_size_bytes", 0)
    jax.config.update("jax_persistent_cache_min_compile_time_secs", 0.0)
except Exception:
    pass

import concourse.tile as tile
from concourse import bacc, mybir
from concourse.bass import ts
from concourse.bass_utils import run_bass_kernel_spmd

F16 = mybir.dt.float16
F32 = mybir.dt.float32
F8 = mybir.dt.float8e4
F8_NP = mybir.dt.np(F8)
AF = mybir.ActivationFunctionType
OP = mybir.AluOpType
PM = mybir.MatmulPerfMode

import os
# precision ladder (all verified on HW via probes; flip off if rel err > 2e-2)
EMBED_F8 = os.environ.get("K_EMBED_F8", "0") == "1"   # video + W_e fp8 DR
EMB_RES = os.environ.get("K_EMBED_RES", "1") == "1"   # residual fp8 embed
XG_F8 = os.environ.get("K_XG_F8", "1") == "1"         # vsb + W_ih fp8 DR
# with XG_F8: gate tiles listed here still use f16 (precision-critical paths)
XG_F16_TILES = os.environ.get("K_XG_F16", "")
# residual fp8: x ~ x8 + xr8, W ~ W8 + Wr8; keep 3 DoubleRow product terms
# (drop Wr*xr) = 0.75x the f16 cost at ~0.3% noise
XG_RES = os.environ.get("K_XG_RES", "1") == "1"
WHH_F8 = os.environ.get("K_WHH_F8", "1") == "1"       # h + W_hh fp8 DR
# g-gate whh via residual fp8 (base fp8 term + h-residual + W-residual)
WHHG_RES = os.environ.get("K_WHHG_RES", "0") == "1"
K_DEBUG = os.environ.get("K_DEBUG", "0") == "1"
WSCALE = 16.0     # fp8 weight pre-scale; 1/WSCALE folded into gate act scale

B, T, F, P, H = 256, 64, 2048, 512, 512
NB = 4            # batch groups
BC = B // NB      # 64 per-core batch
BH = BC // 2      # 32 per-chain batch
TC = 8            # timesteps per phase-A chunk
NCHUNK = T // TC
KF = F // 128     # 16
KP = P // 128     # 4
KH = H // 128     # 4
MG = 4 * H // 128  # 16 gate tiles; order [i(0:4), g(4:8), f(8:12), o(12:16)]
_TILE_RANGES = {"i": range(0, 4), "g": range(4, 8), "f": range(8, 12), "o": range(12, 16)}
XG_F16_SET = set()
for _gn in XG_F16_TILES:
    XG_F16_SET.update(_TILE_RANGES[_gn])
LOOK = 2          # xg lookahead (psg bufs = LOOK + 1)

EMB_DT = F8 if (EMBED_F8 or EMB_RES) else F16
VSB_DT = F8 if XG_F8 else F16
WHH_DT = F8 if WHH_F8 else F16


def build_nc():
    nc = bacc.Bacc("TRN2", target_bir_lowering=False, debug=False, num_devices=8)

    vt_d = nc.dram_tensor("vt", [NCHUNK, 128, KF, TC * BC], EMB_DT, kind="ExternalInput")
    wet_d = nc.dram_tensor("w_et", [128, KF, P], EMB_DT, kind="ExternalInput")
    if EMB_RES:
        vtr_d = nc.dram_tensor("vtr", [NCHUNK, 128, KF, TC * BC], EMB_DT, kind="ExternalInput")
        wetr_d = nc.dram_tensor("w_etr", [128, KF, P], EMB_DT, kind="ExternalInput")
    wih_d = nc.dram_tensor("w_iht", [128, KP, 4 * H], VSB_DT, kind="ExternalInput")
    wb_d = nc.dram_tensor("w_b", [128, 2, 4 * H], VSB_DT, kind="ExternalInput")
    if XG_RES:
        wihr_d = nc.dram_tensor("w_ihtr", [128, KP, 4 * H], VSB_DT, kind="ExternalInput")
    if XG_F8:
        wih16_d = nc.dram_tensor("w_iht16", [128, KP, 4 * H], F16, kind="ExternalInput")
        wb16_d = nc.dram_tensor("w_b16", [128, 4 * H], F16, kind="ExternalInput")
        ones16_d = nc.dram_tensor("ones16", [128, BH], F16, kind="ExternalInput")
    ones_d = nc.dram_tensor("ones2", [128, 2, BH], VSB_DT, kind="ExternalInput")
    whh_d = nc.dram_tensor("w_hht", [128, KH, 4 * H], WHH_DT, kind="ExternalInput")
    whhg_d = nc.dram_tensor("w_hhg", [128, KH, H], F16, kind="ExternalInput")
    if WHHG_RES:
        whhgr_d = nc.dram_tensor("w_hhgr", [128, KH, H], F8, kind="ExternalInput")
    out_d = nc.dram_tensor("out_h", [NCHUNK, 128, TC, KH, BC], F16, kind="ExternalOutput")
    if K_DEBUG:
        dbg_vsb = nc.dram_tensor("dbg_vsb", [128, KP, TC * BC], VSB_DT, kind="ExternalOutput")
        dbg_tht = nc.dram_tensor("dbg_tht", [128, 12, BH], F16, kind="ExternalOutput")
        dbg_tho = nc.dram_tensor("dbg_tho", [128, KH, BH], F16, kind="ExternalOutput")
        dbg_c = nc.dram_tensor("dbg_c", [128, KH, BH], F16, kind="ExternalOutput")

    with tile.TileContext(nc) as tc:
        with (
            tc.tile_pool(name="const", bufs=1) as const,
            tc.tile_pool(name="vload", bufs=2) as vload,
            tc.tile_pool(name="vtp", bufs=2) as vtp,
            tc.tile_pool(name="state", bufs=2) as state,
            tc.tile_pool(name="hout", bufs=2) as hout,
            tc.tile_pool(name="tmp", bufs=2) as tmp,
            tc.tile_pool(name="psg", bufs=LOOK + 1, space="PSUM") as psg,
            tc.tile_pool(name="psv", bufs=1, space="PSUM") as psv,
        ):
            # resident weights: wet first (it gates the first embed);
            # the rest are needed only once the scan starts
            wet = const.tile([128, KF, P], EMB_DT)
            nc.sync.dma_start(wet[:], wet_d.ap())
            if EMB_RES:
                wetr = const.tile([128, KF, P], EMB_DT)
            wih = const.tile([128, KP, 4 * H], VSB_DT)
            wb = const.tile([128, 2, 4 * H], VSB_DT)
            ones2 = const.tile([128, 2, BH], VSB_DT)
            whh = const.tile([128, KH, 4 * H], WHH_DT)
            whhg = const.tile([128, KH, H], F16)
            if WHHG_RES:
                whhgr = const.tile([128, KH, H], F8)
            if XG_RES:
                wihr = const.tile([128, KP, 4 * H], VSB_DT)
            if XG_F8:
                wih16 = const.tile([128, KP, 4 * H], F16)
                wb16 = const.tile([128, 4 * H], F16)
                ones16 = const.tile([128, BH], F16)

            def load_rest():
                nc.sync.dma_start(wih[:], wih_d.ap())
                nc.sync.dma_start(wb[:], wb_d.ap())
                nc.sync.dma_start(ones2[:], ones_d.ap())
                nc.sync.dma_start(whh[:], whh_d.ap())
                nc.sync.dma_start(whhg[:], whhg_d.ap())
                if WHHG_RES:
                    nc.sync.dma_start(whhgr[:], whhgr_d.ap())
                if XG_RES:
                    nc.sync.dma_start(wihr[:], wihr_d.ap())
                if XG_F8:
                    nc.sync.dma_start(wih16[:], wih16_d.ap())
                    nc.sync.dma_start(wb16[:], wb16_d.ap())
                    nc.sync.dma_start(ones16[:], ones16_d.ap())

            # per-chain state (tag-rotated pools)
            def init_state(tag_suffix):
                h8 = state.tile([128, KH, BH], WHH_DT, tag="h8" + tag_suffix)
                nc.gpsimd.memset(h8[:], 0.0)
                cc = state.tile([128, KH, BH], F16, tag="c" + tag_suffix)
                nc.gpsimd.memset(cc[:], 0.0)
                h16 = state.tile([128, KH, BH], F16, tag="h16" + tag_suffix)
                nc.gpsimd.memset(h16[:], 0.0)
                hr8 = None
                if WHHG_RES:
                    hr8 = state.tile([128, KH, BH], F8, tag="hr8" + tag_suffix)
                    nc.gpsimd.memset(hr8[:], 0.0)
                return h8, cc, h16, hr8

            dbg_box = [None]
            h8_prev = {}
            c_prev = {}
            h16_prev = {}
            hr8_prev = {}
            elem_state = {}
            h8_prev[0], c_prev[0], h16_prev[0], hr8_prev[0] = init_state("a")
            h8_prev[1], c_prev[1], h16_prev[1], hr8_prev[1] = init_state("b")

            # ---------------- phase A: embed ----------------
            def embed_items(c):
                """Items producing vsb (fp8, [128, KP, TC*BC]) for chunk c."""
                vch = vload.tile([128, KF, TC * BC], EMB_DT, tag="vch")
                vchr = None
                if EMB_RES:
                    vchr = vload.tile([128, KF, TC * BC], EMB_DT, tag="vchr")
                vsb = vtp.tile([128, KP, TC * BC], VSB_DT, tag="vsb")
                vsb16 = None
                if XG_F8 and XG_F16_SET:
                    vsb16 = vtp.tile([128, KP, TC * BC], F16, tag="vsb16")
                vsbr = None
                if XG_RES:
                    vsbr = vtp.tile([128, KP, TC * BC], VSB_DT, tag="vsbr")

                def dma_item():
                    nc.sync.dma_start(vch[:], vt_d.ap()[c])
                    if EMB_RES:
                        nc.sync.dma_start(vchr[:], vtr_d.ap()[c])

                # fine-grained items: 4-matmul bursts and quarter drains so
                # no single item blocks the scan's PE/DVE queues for long
                pv_box = [None]

                def mm_term(kpl, mp, term, first):
                    # one full-K DoubleRow chain for a residual term:
                    # 0: W8*x8 (arms bank region), 1: W8*xr8, 2: Wr8*x8
                    pv = pv_box[0]
                    lw = wet if term < 2 else wetr
                    mv = vch if term != 1 else vchr
                    for k2 in range(KF // 2):
                        nc.tensor.matmul(
                            pv[:, kpl, :],
                            lw[:, 2 * k2 : 2 * k2 + 2, ts(mp, 128)],
                            mv[:, 2 * k2 : 2 * k2 + 2, :],
                            start=(first and k2 == 0),
                            stop=False,
                            perf_mode=PM.DoubleRow,
                        )

                def mm_quarter(kpl, mp, khalf, first, last):
                    pv = pv_box[0]
                    if EMBED_F8:
                        n2 = KF // 2
                        for k2 in range(khalf * n2 // 2, (khalf + 1) * n2 // 2):
                            nc.tensor.matmul(
                                pv[:, kpl, :],
                                wet[:, 2 * k2 : 2 * k2 + 2, ts(mp, 128)],
                                vch[:, 2 * k2 : 2 * k2 + 2, :],
                                start=(first and k2 == khalf * n2 // 2),
                                stop=(last and k2 == (khalf + 1) * n2 // 2 - 1),
                                perf_mode=PM.DoubleRow,
                            )
                    else:
                        for ko in range(khalf * KF // 2, (khalf + 1) * KF // 2):
                            nc.tensor.matmul(
                                pv[:, kpl, :],
                                wet[:, ko, ts(mp, 128)],
                                vch[:, ko, :],
                                start=(first and ko == khalf * KF // 2),
                                stop=(last and ko == (khalf + 1) * KF // 2 - 1),
                            )

                items = [dma_item]
                if EMB_RES:
                    for kp in (0, 2):
                        def alloc(kp=kp):
                            pv = psv.tile([128, 2, TC * BC], F32, tag="pv", name="pv")
                            pv_box[0] = pv
                            mm_term(0, kp, 0, True)
                        items.append(alloc)
                        for kpl in range(2):
                            mp = kp + kpl
                            if kpl != 0:
                                items.append(lambda kpl=kpl, mp=mp: mm_term(
                                    kpl, mp, 0, True))
                            items.append(lambda kpl=kpl, mp=mp: mm_term(
                                kpl, mp, 1, False))
                            items.append(lambda kpl=kpl, mp=mp: mm_term(
                                kpl, mp, 2, False))
                            def drain(kp=kp, kpl=kpl, mp=mp):
                                nc.vector.tensor_scalar(
                                    vsb[:, mp, :], pv_box[0][:, kpl, :],
                                    1.0 / WSCALE, 0.0, OP.mult, OP.add,
                                )
                            items.append(drain)
                            if vsbr is not None:
                                def drainr(kp=kp, kpl=kpl, mp=mp):
                                    nc.vector.scalar_tensor_tensor(
                                        vsbr[:, mp, :], pv_box[0][:, kpl, :],
                                        1.0 / WSCALE, vsb[:, mp, :],
                                        OP.mult, OP.subtract,
                                    )
                                items.append(drainr)
                            if vsb16 is not None:
                                def drain16(kp=kp, kpl=kpl, mp=mp):
                                    nc.vector.tensor_scalar(
                                        vsb16[:, mp, :], pv_box[0][:, kpl, :],
                                        1.0 / WSCALE, 0.0, OP.mult, OP.add,
                                    )
                                items.append(drain16)
                    return (vsb, vsb16, vsbr), items
                for kp in (0, 2):
                    def alloc(kp=kp):
                        pv = psv.tile([128, 2, TC * BC], F32, tag="pv", name="pv")
                        pv_box[0] = pv
                        mm_quarter(0, kp, 0, True, False)
                    items.append(alloc)
                    for kpl in range(2):
                        mp = kp + kpl
                        if kpl != 0:
                            items.append(lambda kpl=kpl, mp=mp: mm_quarter(
                                kpl, mp, 0, True, False))
                        items.append(lambda kpl=kpl, mp=mp: mm_quarter(
                            kpl, mp, 1, False, True))
                        def drain(kp=kp, kpl=kpl, mp=mp):
                            nc.vector.tensor_scalar(
                                vsb[:, mp, :], pv_box[0][:, kpl, :],
                                1.0 / WSCALE, 0.0, OP.mult, OP.add,
                            )
                        items.append(drain)
                        if vsbr is not None:
                            def drainr(kp=kp, kpl=kpl, mp=mp):
                                # vsbr = v - dequant(vsb8)
                                nc.vector.scalar_tensor_tensor(
                                    vsbr[:, mp, :], pv_box[0][:, kpl, :],
                                    1.0 / WSCALE, vsb[:, mp, :],
                                    OP.mult, OP.subtract,
                                )
                            items.append(drainr)
                        if vsb16 is not None:
                            def drain16(kp=kp, kpl=kpl, mp=mp):
                                nc.vector.tensor_scalar(
                                    vsb16[:, mp, :], pv_box[0][:, kpl, :],
                                    1.0 / WSCALE, 0.0, OP.mult, OP.add,
                                )
                            items.append(drain16)

                return (vsb, vsb16, vsbr), items

            # ---------------- xg into PSUM (per chain) ---------------------
            def emit_xg(t, ch, vsb_map):
                """Open psg accumulation for (step t, chain ch): bias + W_ih@v."""
                pg = psg.tile([128, MG, BH], F32, tag="pg" + ("a" if ch == 0 else "b"))
                vsb, vsb16, vsbr = vsb_map[t // TC]
                tl = t % TC
                lo = tl * BC + ch * BH
                for m in range(MG):
                    # bias first. start=True ONLY on the tile's very first
                    # matmul: start_tensor_calc arms lazy-zero for the WHOLE
                    # 2KB PSUM bank, so arming once lets every region's
                    # first write clear its own bytes and later writes
                    # (xg pairs + whh) accumulate.
                    if XG_F8 and m not in XG_F16_SET:
                        _lab(nc.tensor.matmul(
                            pg[:, m, :],
                            wb[:, :, ts(m, 128)],
                            ones2[:],
                            start=(m == 0), stop=False,
                            perf_mode=PM.DoubleRow,
                        ), f"xgb{m}_c{ch}")
                        for k2 in range(KP // 2):
                            nc.tensor.matmul(
                                pg[:, m, :],
                                wih[:, 2 * k2 : 2 * k2 + 2, ts(m, 128)],
                                vsb[:, 2 * k2 : 2 * k2 + 2, lo : lo + BH],
                                start=False, stop=False,
                                perf_mode=PM.DoubleRow,
                            )
                            if XG_RES:
                                nc.tensor.matmul(
                                    pg[:, m, :],
                                    wih[:, 2 * k2 : 2 * k2 + 2, ts(m, 128)],
                                    vsbr[:, 2 * k2 : 2 * k2 + 2, lo : lo + BH],
                                    start=False, stop=False,
                                    perf_mode=PM.DoubleRow,
                                )
                                nc.tensor.matmul(
                                    pg[:, m, :],
                                    wihr[:, 2 * k2 : 2 * k2 + 2, ts(m, 128)],
                                    vsb[:, 2 * k2 : 2 * k2 + 2, lo : lo + BH],
                                    start=False, stop=False,
                                    perf_mode=PM.DoubleRow,
                                )
                    elif XG_F8:
                        # f16 path for precision-critical gate tiles
                        _lab(nc.tensor.matmul(
                            pg[:, m, :], wb16[:, ts(m, 128)], ones16[:],
                            start=(m == 0), stop=False,
                        ), f"xgb{m}_c{ch}")
                        for kp in range(KP):
                            nc.tensor.matmul(
                                pg[:, m, :],
                                wih16[:, kp, ts(m, 128)],
                                vsb16[:, kp, lo : lo + BH],
                                start=False, stop=False,
                            )
                    else:
                        nc.tensor.matmul(
                            pg[:, m, :], wb[:, 0, ts(m, 128)], ones2[:, 0, :],
                            start=(m == 0), stop=False,
                        )
                        for kp in range(KP):
                            nc.tensor.matmul(
                                pg[:, m, :],
                                wih[:, kp, ts(m, 128)],
                                vsb[:, kp, lo : lo + BH],
                                start=False, stop=False,
                            )
                return pg

            # ---------------- scan step for one chain ---------------------
            def emit_whh(ch, pg):
                """W_hh @ h8 accumulation for chain ch into its pg.

                f tiles (12:16) first so the sigmoid act unblocks earliest.
                """
                h8 = h8_prev[ch]
                h16 = h16_prev[ch]
                sfx = "a" if ch == 0 else "b"
                for m in range(MG):  # o tiles (12:16) naturally last
                    if WHH_F8 and not 4 <= m < 8:
                        for k2 in range(KH // 2):
                            _lab(nc.tensor.matmul(
                                pg[:, m, :],
                                whh[:, 2 * k2 : 2 * k2 + 2, ts(m, 128)],
                                h8[:, 2 * k2 : 2 * k2 + 2, :],
                                start=False,
                                stop=(k2 == KH // 2 - 1),
                                perf_mode=PM.DoubleRow,
                            ), f"whh{m}k{k2}_" + sfx)
                    elif WHH_F8 and WHHG_RES:
                        # g tiles via residual fp8: U8*h8 + U8*hr8 + Ur8*h8
                        hr8 = hr8_prev[ch]
                        for k2 in range(KH // 2):
                            for lw, mv in ((whh, h8), (whh, hr8), (whhgr, h8)):
                                lsl = (
                                    lw[:, 2 * k2 : 2 * k2 + 2, ts(m, 128)]
                                    if lw is whh
                                    else lw[:, 2 * k2 : 2 * k2 + 2, ts(m - 4, 128)]
                                )
                                _lab(nc.tensor.matmul(
                                    pg[:, m, :],
                                    lsl,
                                    mv[:, 2 * k2 : 2 * k2 + 2, :],
                                    start=False,
                                    stop=(k2 == KH // 2 - 1 and mv is h8 and lw is not whh),
                                    perf_mode=PM.DoubleRow,
                                ), f"whh{m}k{k2}_" + sfx)
                    elif WHH_F8:
                        # g tiles in f16 (full-slope tanh path needs precision)
                        for kh in range(KH):
                            _lab(nc.tensor.matmul(
                                pg[:, m, :],
                                whhg[:, kh, ts(m - 4, 128)],
                                h16[:, kh, :],
                                start=False,
                                stop=(kh == KH - 1),
                            ), f"whh{m}k{kh}_" + sfx)
                    else:
                        for kh in range(KH):
                            nc.tensor.matmul(
                                pg[:, m, :],
                                whh[:, kh, ts(m, 128)],
                                h8[:, kh, :],
                                start=False,
                                stop=(kh == KH - 1),
                            )

            def emit_elem(ch, pg, tl, hstage):
                """acts + vector ops for chain ch's step; updates h8/c.

                Gate tiles: i 0:4, g 4:8, o 8:12, f 12:16. i and o rows are
                pre-halved on the host so one tanh covers sigmoid-via-tanh
                for i/o plus the real tanh for g. The recurrence carries
                h' = 2h = (th_o+1)*tanh(c); W_hh is pre-scaled x0.5 and the
                host halves the staged output.
                """
                lo = ch * BH
                sfx = "a" if ch == 0 else "b"
                tht = tmp.tile([128, 12, BH], F16, tag="tht" + sfx)
                tho = tmp.tile([128, KH, BH], F16, tag="tho" + sfx)
                # chain act: tanh over [i, g, f] (i/f pre-halved)
                _lab(nc.scalar.activation(
                    tht[:], pg[:, 0:12, :], AF.Tanh,
                    scale=1.0 / WSCALE,
                ), "act1_" + sfx)
                # off-chain act: tanh over [o] (pre-halved); needed only at h8
                _lab(nc.scalar.activation(
                    tho[:], pg[:, 12:16, :], AF.Tanh,
                    scale=1.0 / WSCALE,
                ), "acto_" + sfx)
                # state is C' = 2c:
                #   m2' = (th_i+1)*th_g = 2*s_i*t_g
                #   m1' = (th_f+1)*C'   = 4*s_f*c
                #   C'  = 0.5*m1' + m2' = 2*(s_f*c + s_i*t_g)
                if ch == 0:
                    dbg_box[0] = tht
                m2 = tmp.tile([128, KH, BH], F16, tag="m2" + sfx)
                _lab(nc.vector.scalar_tensor_tensor(
                    m2[:], tht[:, 0:4, :], 1.0, tht[:, 4:8, :], OP.add, OP.mult
                ), "m2_" + sfx)
                m1 = tmp.tile([128, KH, BH], F16, tag="m1" + sfx)
                _lab(nc.vector.scalar_tensor_tensor(
                    m1[:], tht[:, 8:12, :], 1.0, c_prev[ch][:], OP.add, OP.mult
                ), "m1_" + sfx)
                c_new = state.tile([128, KH, BH], F16, tag="c" + sfx)
                _lab(nc.vector.scalar_tensor_tensor(
                    c_new[:], m1[:], 0.5, m2[:], OP.mult, OP.add
                ), "c_" + sfx)
                elem_state[ch] = (tho, c_new)
                c_prev[ch] = c_new

            def emit_tail(ch, tl, hstage):
                lo = ch * BH
                sfx = "a" if ch == 0 else "b"
                tho, c_new = elem_state[ch]
                tc_t = tmp.tile([128, KH, BH], F16, tag="tc" + sfx)
                # tanh(c) = tanh(0.5 * C')
                _lab(nc.scalar.activation(tc_t[:], c_new[:], AF.Tanh, scale=0.5),
                     "tanhc_" + sfx)
                # h' = 2h = (th_o+1)*tanh(c)
                h8_new = state.tile([128, KH, BH], WHH_DT, tag="h8" + sfx)
                _lab(nc.vector.scalar_tensor_tensor(
                    h8_new[:], tho[:], 1.0, tc_t[:], OP.add, OP.mult
                ), "h8_" + sfx)
                h16_new = state.tile([128, KH, BH], F16, tag="h16" + sfx)
                _lab(nc.vector.scalar_tensor_tensor(
                    h16_new[:], tho[:], 1.0, tc_t[:], OP.add, OP.mult
                ), "h16_" + sfx)
                _lab(nc.vector.tensor_scalar(
                    hstage[:, tl, :, lo : lo + BH], h16_new[:], 1.0, 0.0,
                    OP.mult, OP.add
                ), "hs_" + sfx)
                if WHHG_RES:
                    hr8_new = state.tile([128, KH, BH], F8, tag="hr8" + sfx)
                    _lab(nc.vector.tensor_tensor(
                        hr8_new[:], h16_new[:], h8_new[:], OP.subtract
                    ), "hr8_" + sfx)
                    hr8_prev[ch] = hr8_new
                h8_prev[ch] = h8_new
                h16_prev[ch] = h16_new

            # ---------------- main pipeline --------------------------------
            vsb_map = {}
            vsb_map[0], items0 = embed_items(0)
            items0[0]()       # vt(+vtr) chunk-0 DMA right behind wet
            if EMB_RES:
                nc.sync.dma_start(wetr[:], wetr_d.ap())
            load_rest()       # remaining weights overlap the first embed
            for it in items0[1:]:
                it()
            if K_DEBUG:
                nc.sync.dma_start(dbg_vsb.ap(), vsb_map[0][0][:])
            pg_q = {}
            for t in range(LOOK):
                for ch in (0, 1):
                    pg_q[(t, ch)] = emit_xg(t, ch, vsb_map)

            for c in range(NCHUNK):
                if c + 1 < NCHUNK:
                    vsb_map[c + 1], items = embed_items(c + 1)
                else:
                    items = []
                hstage = hout.tile([128, TC, KH, BC], F16, tag="hs")
                n_items = len(items)
                emitted = 0
                for tl in range(TC):
                    t = c * TC + tl
                    pgs = [pg_q.pop((t, ch)) for ch in (0, 1)]
                    for ch in (0, 1):
                        emit_whh(ch, pgs[ch])
                    for ch in (0, 1):
                        emit_elem(ch, pgs[ch], tl, hstage)
                    if K_DEBUG and t == 0:
                        nc.sync.dma_start(dbg_tht.ap(), dbg_box[0][:])
                        tho0, c0 = elem_state[0]
                        nc.sync.dma_start(dbg_tho.ap(), tho0[:])
                        nc.sync.dma_start(dbg_c.ap(), c0[:])
                    for ch in (0, 1):
                        emit_tail(ch, tl, hstage)
                    # embed items for next chunk: ALL must be emitted
                    # before any xg that reads the next chunk's vsb
                    want = min(n_items, (n_items * (tl + 1) + TC - LOOK - 1) // (TC - LOOK))
                    while emitted < want:
                        items[emitted]()
                        emitted += 1
                    # xg lookahead after the critical whh/elem emissions
                    if t + LOOK < T:
                        for ch in (0, 1):
                            pg_q[(t + LOOK, ch)] = emit_xg(t + LOOK, ch, vsb_map)
                nc.sync.dma_start(out_d.ap()[c], hstage[:])
                vsb_map.pop(c, None)

    nc.compile()
    return nc


LABELS = {}


def _lab(inst, label):
    try:
        LABELS[inst.ins.name] = label
    except Exception:
        pass
    return inst


_CACHED_NC = None


def _get_nc():
    global _CACHED_NC
    if _CACHED_NC is None:
        _CACHED_NC = build_nc()
    return _CACHED_NC


def _prep_inputs(video_feats, W_e, b_e, W_ih1, W_hh1, b_ih1, b_hh1,
                 W_ih2, W_hh2, b_ih2, b_hh2):
    """Build the 8 per-core input maps (host-side shard + layout prep)."""
    # gate permutation [i, f, g, o] -> [i, g, f, o]; i/f/o rows scaled 0.5
    # so one tanh covers sigmoid-via-tanh for them; g stays a real tanh.
    perm = np.concatenate(
        [
            np.arange(0 * H, 1 * H),  # i
            np.arange(2 * H, 3 * H),  # g
            np.arange(1 * H, 2 * H),  # f
            np.arange(3 * H, 4 * H),  # o
        ]
    )
    s = np.ones((4 * H,), np.float32)
    s[0 * H : 1 * H] = 0.5  # i rows (post-perm)
    s[2 * H : 3 * H] = 0.5  # f rows (post-perm)
    s[3 * H : 4 * H] = 0.5  # o rows (post-perm)

    emb_np = F8_NP if (EMBED_F8 or EMB_RES) else np.float16
    vsb_np = F8_NP if XG_F8 else np.float16
    whh_np = F8_NP if WHH_F8 else np.float16

    wet32 = W_e.T * WSCALE
    wet = np.ascontiguousarray(
        wet32.astype(emb_np).reshape(KF, 128, P).transpose(1, 0, 2)
    )
    wetr = None
    if EMB_RES:
        wr = wet32 - wet32.astype(F8_NP).astype(np.float32)
        wetr = np.ascontiguousarray(
            wr.astype(emb_np).reshape(KF, 128, P).transpose(1, 0, 2)
        )

    per_dir = []
    for (W_ih, W_hh, b_ih, b_hh) in (
        (W_ih1, W_hh1, b_ih1, b_hh1),
        (W_ih2, W_hh2, b_ih2, b_hh2),
    ):
        # permuted+scaled gate rows, transposed to [K, 4H], x WSCALE.
        # W_hh gets an extra x0.5: its rhs is h' = 2h.
        wp = (W_ih[perm] * s[:, None]).T * WSCALE
        wh = (W_hh[perm] * s[:, None]).T * (WSCALE * 0.5)
        bb = ((b_ih + b_hh + W_ih @ b_e)[perm] * s) * WSCALE
        wb = np.zeros((128, 2, 4 * H), np.float32)
        wb[0, 0, :] = bb
        wb16f = np.zeros((128, 4 * H), np.float16)
        wb16f[0, :] = bb.astype(np.float16)
        wpr = wp - wp.astype(F8_NP).astype(np.float32)
        per_dir.append(
            (
                np.ascontiguousarray(
                    wpr.astype(vsb_np).reshape(KP, 128, 4 * H).transpose(1, 0, 2)
                ),
                np.ascontiguousarray(
                    wp.astype(vsb_np).reshape(KP, 128, 4 * H).transpose(1, 0, 2)
                ),
                wb.astype(vsb_np),
                np.ascontiguousarray(
                    wh.astype(whh_np).reshape(KH, 128, 4 * H).transpose(1, 0, 2)
                ),
                np.ascontiguousarray(
                    wh[:, H : 2 * H].astype(np.float16).reshape(KH, 128, H).transpose(1, 0, 2)
                ),
                np.ascontiguousarray(
                    (wh[:, H : 2 * H] - wh[:, H : 2 * H].astype(F8_NP).astype(np.float32))
                    .astype(F8_NP).reshape(KH, 128, H).transpose(1, 0, 2)
                ),
                np.ascontiguousarray(
                    wp.astype(np.float16).reshape(KP, 128, 4 * H).transpose(1, 0, 2)
                ),
                wb16f,
            )
        )

    ones2 = np.zeros((128, 2, BH), np.float32)
    ones2[0, 0, :] = 1.0
    ones2 = ones2.astype(vsb_np)
    ones16 = np.zeros((128, BH), np.float16)
    ones16[0, :] = 1.0

    # videoT [F, T, B] -> fwd + time-reversed
    vt32 = np.ascontiguousarray(video_feats.transpose(2, 1, 0))
    vt_full = vt32.astype(emb_np)
    vt_rev = np.ascontiguousarray(vt_full[:, ::-1, :])
    vtr_full = vtr_rev = None
    if EMB_RES:
        vtr_full = (vt32 - vt_full.astype(np.float32)).astype(emb_np)
        vtr_rev = np.ascontiguousarray(vtr_full[:, ::-1, :])

    in_maps = []
    for core in range(8):
        g, d = divmod(core, 2)
        src = vt_full if d == 0 else vt_rev
        vt = np.ascontiguousarray(
            src[:, :, g * BC : (g + 1) * BC]
            .reshape(KF, 128, NCHUNK, TC * BC)
            .transpose(2, 1, 0, 3)
        )
        if EMB_RES:
            srcr = vtr_full if d == 0 else vtr_rev
            vtr = np.ascontiguousarray(
                srcr[:, :, g * BC : (g + 1) * BC]
                .reshape(KF, 128, NCHUNK, TC * BC)
                .transpose(2, 1, 0, 3)
            )
        wihrp, wihp, wbp, whhp, whhgp, whhgrp, wihp16, wbp16 = per_dir[d]
        im = {
            "vt": vt,
            "w_et": wet,
            "w_iht": wihp,
            "w_b": wbp,
            "ones2": ones2,
            "w_hht": whhp,
            "w_hhg": whhgp,
        }
        if WHHG_RES:
            im["w_hhgr"] = whhgrp
        if EMB_RES:
            im["vtr"] = vtr
            im["w_etr"] = wetr
        if XG_RES:
            im["w_ihtr"] = wihrp
        if XG_F8:
            im["w_iht16"] = wihp16
            im["w_b16"] = wbp16
            im["ones16"] = ones16
        in_maps.append(im)
    return in_maps


last_exec_ns = None
last_wall_s = None


def kernel(**inputs):
    global last_exec_ns, last_wall_s
    nc = _get_nc()
    inputs = {k: np.asarray(v, dtype=np.float32) for k, v in inputs.items()}
    in_maps = _prep_inputs(**inputs)
    t0 = time.perf_counter()
    res = run_bass_kernel_spmd(nc, in_maps, core_ids=list(range(8)))
    last_wall_s = time.perf_counter() - t0
    last_exec_ns = res.exec_time_ns

    lstm1 = np.empty((B, T, H), np.float32)
    lstm2 = np.empty((B, T, H), np.float32)
    for core in range(8):
        g, d = divmod(core, 2)
        oh = res.results[core]["out_h"]  # [NCHUNK, 128, TC, KH, BC] f16, = 2h
        h = np.transpose(oh.astype(np.float32), (4, 0, 2, 3, 1)).reshape(BC, T, H)
        h *= 0.5
        if d == 0:
            lstm1[g * BC : (g + 1) * BC] = h
        else:
            lstm2[g * BC : (g + 1) * BC] = h[:, ::-1, :]
    return (lstm1, lstm2)
